# revision 4
# baseline (speedup 1.0000x reference)
"""Trainium2 Bass kernel for ColaViT pre-attention QKV down-projection.

Computes gelu(hidden_states @ concat(w_q, w_k, w_v)) and splits into
(q_low, k_low, v_low), matching the fp32 jax reference.

Sharding: data-parallel on batch across 8 NeuronCores; each core owns
M=1576 token rows of the [12608, 768] x [768, 576] GEMM + exact Gelu.

v3 strategy (from trace analysis of v1/v2):
- All inputs host-packed into contiguous fp16 buffers so every DMA is
  a full-rate 2D copy. Loads on the sync HWDGE ring, stores on the
  scalar HWDGE ring (no head-of-line blocking).
- PE warm-up sized to bridge from program start to first-data-ready so
  the HAM clock gate flips to 2.4 GHz right as the real stream begins
  (v2 lost ~5us streaming at 1.2 GHz because warm-up was too short and
  the early stream too sparse).
- Stationary = x m-tile, moving = w [128,288]; per m-tile the k loop is
  split k0-2/k3-5 so only w-half0 gates the start; PSUM tiles span two
  banks so one exact-Gelu ACTIVATE per m-tile evicts both n-halves
  (fewer ACT fixed costs: (N+352)/1.2 ns each).
- The 40-row tail chunk (slow, unaligned store descriptor ~1.3us) is
  processed SECOND so its latency hides under the stream; the last
  chunk is a single fast 128-row tile, minimizing last-MM -> final-
  barrier latency. The NRT postamble (51 sem resets/engine, ~7us) is
  fixed cost we cannot remove.
- fp16 outputs; host un-permutes and casts to fp32.
"""

import numpy as np

HIDDEN = 768
RANK = 192
N_OUT = 3 * RANK          # 576
B, S = 64, 197
N_CORES = 8
M_PER_CORE = B * S // N_CORES   # 1576
P = 128
K_TILES = HIDDEN // P     # 6
N_CHUNK = 288             # one n-half
N_WARMUP_MM = 7

# chunks in PROCESSING order: (row_offset, rows). 40-row tail second.
CHUNKS = [(0, 2 * P), (1536, 40), (256, 3 * P), (640, 4 * P),
          (1152, 2 * P), (1408, P)]
assert sum(c[1] for c in CHUNKS) == M_PER_CORE

_CACHE = {}


def _build_nc():
    from contextlib import ExitStack

    import concourse.bacc as bacc
    import concourse.mybir as mybir
    from concourse.tile import TileContext

    f32 = mybir.dt.float32
    f16 = mybir.dt.float16
    gelu = mybir.ActivationFunctionType.Gelu

    nc = bacc.Bacc("TRN2", target_bir_lowering=False, debug=False,
                   num_devices=N_CORES)

    w_dram = [nc.dram_tensor(f"w{h}", [P, 3 * N_OUT], f16,
                             kind="ExternalInput") for h in range(2)]
    x_dram = [nc.dram_tensor(f"x{ci}", [P, K_TILES * csz], f16,
                             kind="ExternalInput")
              for ci, (_, csz) in enumerate(CHUNKS)]
    y_dram = []
    for ci, (_, csz) in enumerate(CHUNKS):
        if csz % P == 0:
            y_dram.append(nc.dram_tensor(f"y{ci}", [P, (csz // P) * N_OUT],
                                         f16, kind="ExternalOutput"))
        else:
            y_dram.append(nc.dram_tensor(f"y{ci}", [csz, N_OUT], f16,
                                         kind="ExternalOutput"))

    with TileContext(nc) as tc, ExitStack() as ctx:
        sb = ctx.enter_context(tc.tile_pool(name="sb", bufs=1))
        pp = ctx.enter_context(tc.tile_pool(name="pp", bufs=3, space="PSUM"))

        # PE warm-up: zero tile memset on gpsimd (free early), then a
        # burst of matmuls that keeps the PE busy until first data
        # lands (~3.3us) so HAM un-throttles right as the stream starts.
        zt = sb.tile([P, 520], f16, tag="zt", name="zt")
        nc.gpsimd.memset(zt[:], 0.0)
        zps = pp.tile([8, 512], f32, tag="zps", name="zps", bufs=1)
        for _ in range(N_WARMUP_MM):
            nc.tensor.matmul(zps[:], zt[:, :8], zt[:, 8:520],
                             start=True, stop=True)

        # loads on the sync HWDGE ring; w-half0 first so compute can
        # start, w-half1 after the first x chunk.
        wt = [sb.tile([P, 3, N_OUT], f16, tag=f"w{h}", name=f"w{h}")
              for h in range(2)]
        xt = [sb.tile([P, K_TILES, csz], f16, tag=f"x{ci}", name=f"x{ci}")
              for ci, (_, csz) in enumerate(CHUNKS)]
        nc.sync.dma_start(wt[0][:], w_dram[0][:].rearrange(
            "p (a n) -> p a n", a=3))
        nc.sync.dma_start(xt[0][:], x_dram[0][:].rearrange(
            "p (a m) -> p a m", a=K_TILES))
        nc.sync.dma_start(wt[1][:], w_dram[1][:].rearrange(
            "p (a n) -> p a n", a=3))
        for ci in range(1, len(CHUNKS)):
            nc.sync.dma_start(xt[ci][:], x_dram[ci][:].rearrange(
                "p (a m) -> p a m", a=K_TILES))

        for ci, (c0, csz) in enumerate(CHUNKS):
            n_mt = (csz + P - 1) // P
            ysb = sb.tile([P, n_mt, N_OUT], f16, tag=f"ysb{ci}",
                          name=f"ysb{ci}")
            for mj in range(n_mt):
                msz = min(P, csz - mj * P)
                ml = mj * P
                # one 2-bank PSUM tile per m-tile: bank nj holds n-half nj
                ps = pp.tile([P, 2, 512], f32, tag="ps",
                             name=f"ps{ci}_{mj}")
                # k-split: w-half0 gates the first 6 matmuls only
                for kh in range(2):
                    for nj in range(2):
                        for kk in range(3):
                            k = kh * 3 + kk
                            nc.tensor.matmul(
                                ps[:msz, nj, :N_CHUNK],
                                xt[ci][:, k, ml:ml + msz],
                                wt[kh][:, kk, nj * N_CHUNK:(nj + 1) * N_CHUNK],
                                start=(k == 0),
                                stop=(k == K_TILES - 1),
                            )
                # single exact-Gelu eviction of both n-halves -> fp16
                nc.scalar.activation(ysb[:msz, mj, :],
                                     ps[:msz, :, :N_CHUNK], gelu)
            if csz % P == 0:
                nc.scalar.dma_start(
                    y_dram[ci][:].rearrange("p (a n) -> p a n", a=n_mt),
                    ysb[:, :, :])
            else:
                nc.scalar.dma_start(y_dram[ci][:, :], ysb[:csz, 0, :])

    nc.compile()
    return nc


def _get_nc():
    if "nc" not in _CACHE:
        _CACHE["nc"] = _build_nc()
    return _CACHE["nc"]


def _make_in_maps(hidden_states, w_q, w_k, w_v):
    x = np.asarray(hidden_states, dtype=np.float32).reshape(B * S, HIDDEN)
    xT16 = np.ascontiguousarray(x.T).astype(np.float16)     # [768, 12608]
    wcat = np.concatenate(
        [np.asarray(w_q, np.float32), np.asarray(w_k, np.float32),
         np.asarray(w_v, np.float32)], axis=1).astype(np.float16)
    w_pack = []
    for h in range(2):
        seg = wcat[h * 3 * P:(h + 1) * 3 * P, :].reshape(3, P, N_OUT)
        w_pack.append(np.ascontiguousarray(
            seg.transpose(1, 0, 2).reshape(P, 3 * N_OUT)))

    in_maps = []
    for c in range(N_CORES):
        base = c * M_PER_CORE
        m = {f"w{h}": w_pack[h] for h in range(2)}
        for ci, (c0, csz) in enumerate(CHUNKS):
            seg = xT16[:, base + c0:base + c0 + csz]        # [768, csz]
            seg = seg.reshape(K_TILES, P, csz).transpose(1, 0, 2)
            m[f"x{ci}"] = np.ascontiguousarray(
                seg.reshape(P, K_TILES * csz))
        in_maps.append(m)
    return in_maps


def _postprocess(results):
    y_full = np.empty((B * S, N_OUT), dtype=np.float32)
    for c in range(N_CORES):
        base = c * M_PER_CORE
        res = results[c]
        for ci, (c0, csz) in enumerate(CHUNKS):
            buf = res[f"y{ci}"]
            if csz % P == 0:
                n_mt = csz // P
                seg = buf.reshape(P, n_mt, N_OUT).transpose(1, 0, 2)
                y_full[base + c0:base + c0 + csz, :] = \
                    seg.reshape(csz, N_OUT)
            else:
                y_full[base + c0:base + c0 + csz, :] = buf
    y_full = y_full.reshape(B, S, N_OUT)
    q = np.ascontiguousarray(y_full[:, :, :RANK])
    k = np.ascontiguousarray(y_full[:, :, RANK:2 * RANK])
    v = np.ascontiguousarray(y_full[:, :, 2 * RANK:])
    return (q, k, v)


def kernel(hidden_states, w_q, w_k, w_v):
    from concourse.bass_utils import run_bass_kernel_spmd

    nc = _get_nc()
    in_maps = _make_in_maps(hidden_states, w_q, w_k, w_v)
    res = run_bass_kernel_spmd(nc, in_maps, list(range(N_CORES)))
    return _postprocess(res.results)
